# revision 9
# baseline (speedup 1.0000x reference)
"""ConvProduct forward (one-hot 2x2/stride-2 conv) as a Bass/Tile kernel on 8 trn2 cores.

Pure data parallel over batch (8 batches/core). Per batch:
  - SWDGE cast-DMA x[b] f32 -> Q [128, 2048] bf16, partition p = kh*64 + ho,
    free (w, cin); SWDGE spreads descriptors across all 16 SDMA engines.
  - one DVE 32x32 block-transpose Q -> T (bf16): T[32A+i, 32B+j] with
    A = kh*2+a holds x[b, 2*(32a+j)+kh, (2B+kw)*16+cin], i = kw*16+cin;
    col block B = wo.
  - the transpose source is block-permuted so T col-block (c*4+wg) holds
    wo = 16*wg+c; per c in 0..15: ONE bf16 matmul, full-array stationary
    lhsT = T[:, c*128:(c+1)*128] (m = wg*32+j, pixel (ho=a*32+j, wo=16wg+c)),
    moving rhs = W2 [128, 512] whose column halves are the a=0 / a=1 masked
    one-hot gathers. The K=128 contraction covers both kh strips and
    zero-masks the wrong a-half, so psum[m, a*256+o] is complete in one N=512
    matmul; the 128-col LDWEIGHTS is amortized over both ho-halves.
  - evacuation: psum c-pair (c0,c1) -> st_a[:, cp*512:...] with a strided-src
    [128,512] copy, alternating ScalarE/VectorE.
  - store: per (b, a) four HWDGE DMAs (one per wg partition quarter), each
    [j=32, (c o)]: every partition is one fully contiguous 16KB run and the
    outermost dim (32) spreads descriptors across the 16 SDMA engines;
    alternate sync/scalar rings per a.
"""
import numpy as np

B, H, Wd, Cin = 64, 128, 128, 16
KH, KW, Cout = 2, 2, 256
Ho, Wo = 64, 64
NCORES = 8
BPC = B // NCORES

_CACHE = {}


def _build_nc_v3():
    import concourse.bass as bass
    import concourse.mybir as mybir
    import concourse.tile as tile
    from concourse import bacc

    f32 = mybir.dt.float32
    bf16 = mybir.dt.bfloat16
    nc = bacc.Bacc("TRN2", target_bir_lowering=False, debug=False)

    x = nc.dram_tensor("x", [BPC, H, Wd, Cin], f32, kind="ExternalInput")
    w = nc.dram_tensor("w", [128, 2 * Cout], bf16, kind="ExternalInput")
    out = nc.dram_tensor("out", [BPC, Ho, Wo, Cout], f32, kind="ExternalOutput")

    with tile.TileContext(nc) as tc:
        with (
            tc.tile_pool(name="wp", bufs=1) as wp,
            tc.tile_pool(name="qp", bufs=3) as qp,
            tc.tile_pool(name="tp", bufs=2) as tp,
            tc.tile_pool(name="sp", bufs=4) as sp,
            tc.tile_pool(name="pp", bufs=4, space="PSUM") as pp,
        ):
            w_sb = wp.tile([128, 2 * Cout], bf16)
            nc.sync.dma_start(w_sb[:], w.ap())

            for b in range(BPC):
                q = qp.tile([128, Wd * Cin], bf16, tag="q")
                src = x.ap()[b].rearrange("(ho kh) w c -> kh ho (w c)", kh=2)
                nc.gpsimd.dma_start(q[:], src)

                t = tp.tile([128, Wd * Cin], bf16, tag="t")
                # permuted-source block transpose: t col-block (c*4+wg) is the
                # in-block transpose of q col-block B = 16*wg+c (= wo), so each
                # c-group's 128 columns cover wo in {c, 16+c, 32+c, 48+c}
                qperm = q[:].rearrange("p (wg c e) -> p c wg e", wg=4, c=16, e=32)
                tv = t[:].rearrange("p (c wg e) -> p c wg e", wg=4, c=16, e=32)
                nc.vector.transpose(tv, qperm)

                sts = [
                    sp.tile([128, 16 * Cout], f32, tag=f"st{a}", name=f"st{a}_{b}")
                    for a in range(2)
                ]
                for cp in range(8):
                    pt = pp.tile([128, 1024], f32, tag="ps")
                    for cc in range(2):
                        c = 2 * cp + cc
                        nc.tensor.matmul(
                            pt[:, cc * 512:(cc + 1) * 512],
                            t[:, c * 128:(c + 1) * 128],
                            w_sb[:],
                            start=True,
                            stop=True,
                        )
                    # evac: (c0,c1) x one a-half per op, strided src over the
                    # two banks
                    pv = pt[:].rearrange("p (cc a o) -> p a cc o", cc=2, a=2)
                    for a in range(2):
                        stsl = sts[a][:, cp * 512:(cp + 1) * 512].rearrange(
                            "p (cc o) -> p cc o", cc=2
                        )
                        if (cp + a) % 2 == 0:
                            nc.scalar.copy(stsl, pv[:, a])
                        else:
                            nc.vector.tensor_copy(stsl, pv[:, a])

                dsts = out.ap()[b].rearrange(
                    "(a j) (wg c) o -> a wg j (c o)", a=2, wg=4
                )
                for a in range(2):
                    eng = nc.sync if a == 0 else nc.scalar
                    for wg in range(4):
                        eng.dma_start(
                            dsts[a][wg], sts[a][wg * 32:(wg + 1) * 32, :]
                        )

    nc.compile()
    return nc


def _get_nc():
    if "v3" not in _CACHE:
        _CACHE["v3"] = _build_nc_v3()
    return _CACHE["v3"]


def _build_w(kernel_idx: np.ndarray) -> np.ndarray:
    import ml_dtypes

    kidx = np.asarray(kernel_idx).astype(np.int64)
    w = np.zeros((128, 2 * Cout), np.float32)
    o = np.arange(Cout)
    for kh in range(KH):
        for a in range(2):
            for kw in range(KW):
                w[kh * 64 + a * 32 + kw * 16 + kidx[kh, kw], a * Cout + o] = 1.0
    return w.astype(ml_dtypes.bfloat16)


def kernel(x: np.ndarray, kernel_idx: np.ndarray) -> np.ndarray:
    from concourse.bass_utils import run_bass_kernel_spmd

    x = np.ascontiguousarray(np.asarray(x, dtype=np.float32))
    w = _build_w(kernel_idx)
    nc = _get_nc()

    in_maps = [
        {"x": x[c * BPC:(c + 1) * BPC], "w": w} for c in range(NCORES)
    ]
    res = run_bass_kernel_spmd(nc, in_maps, core_ids=list(range(NCORES)))
    kernel.last_results = res
    return np.concatenate([res.results[c]["out"] for c in range(NCORES)], axis=0)
